# revision 4
# baseline (speedup 1.0000x reference)
"""Trainium2 Bass kernel for a 16-head dense attention layer.

Problem: x[1,4096,1024] @ w_qkv[1024,3072] -> 16-head attention (N=4096,
D=64) -> @ w_out[1024,1024].

Sharding: tensor-parallel over heads across 8 NeuronCores (2 heads/core).
Each core computes q/k/v^T for its 2 heads (weights column-sliced on host),
attention with a fused, max-free softmax (scores are bounded, inputs are
xavier/randn scaled, so exp never overflows in fp32; denominator comes from
an appended ones-column in V), then an AllToAll converts the head-sharded
attention output into a sequence-sharded layout so every core applies the
full output projection to its own 512 rows. Host concatenates the 8 row
slices.

Matmuls run as float32r (fp32 data reinterpreted; 1 PE cycle/row at free
dim >= 256) by default; set TRN_MM_MODE=f32 for full-precision fp32 (4x
slower).
"""

import os
import numpy as np

N_CORES = 8
N = 4096
HIDDEN = 1024
D = 64
HPC = 2  # heads per core
AD = HPC * D  # 128 att-dim rows per core
NT = N // 128  # 32 n-tiles of 128 (k-tiles of attention)
HT = HIDDEN // 128  # 8 hidden tiles
QCHUNK = 1024
NQC = N // QCHUNK  # 4 q-chunks
NSLICE = N // N_CORES  # 512 rows of output per core

_CACHE = {}


def _build(mm_mode: str):
    import concourse.bass as bass
    import concourse.mybir as mybir
    import concourse.tile as tile
    from concourse import bacc
    from concourse.masks import make_identity

    DT = mybir.dt.float32
    DTR = {"f32r": mybir.dt.float32r, "f32": mybir.dt.float32}[mm_mode]

    AF = mybir.ActivationFunctionType

    nc = bacc.Bacc("TRN2", debug=False, num_devices=N_CORES)

    xT = nc.dram_tensor("xT", [HIDDEN, N], DTR, kind="ExternalInput").ap()
    wq = nc.dram_tensor("wq", [HIDDEN, AD], DTR, kind="ExternalInput").ap()
    wk = nc.dram_tensor("wk", [HIDDEN, AD], DTR, kind="ExternalInput").ap()
    wv = nc.dram_tensor("wv", [HIDDEN, AD], DTR, kind="ExternalInput").ap()
    bq = nc.dram_tensor("bq", [AD, 1], DT, kind="ExternalInput").ap()
    bk = nc.dram_tensor("bk", [AD, 1], DT, kind="ExternalInput").ap()
    bv = nc.dram_tensor("bv", [AD, 1], DT, kind="ExternalInput").ap()
    wo = nc.dram_tensor("wo", [HIDDEN, HIDDEN], DTR, kind="ExternalInput").ap()
    bo = nc.dram_tensor("bo", [1, HIDDEN], DT, kind="ExternalInput").ap()
    out = nc.dram_tensor("out", [NSLICE, HIDDEN], DT, kind="ExternalOutput").ap()

    with tile.TileContext(nc) as tc:
        with (
            tc.tile_pool(name="sb", bufs=1) as sb,
            tc.tile_pool(name="ps", bufs=2, space="PSUM") as ps,
            tc.tile_pool(name="dram", bufs=1, space="DRAM") as dram,
        ):
            # ---- static tiles -------------------------------------------
            ident = sb.tile([128, 128], DT)
            make_identity(nc, ident)

            wq_sb = [sb.tile([128, AD], DTR, name=f"wq{i}", tag="w", bufs=24) for i in range(HT)]
            wk_sb = [sb.tile([128, AD], DTR, name=f"wk{i}", tag="w", bufs=24) for i in range(HT)]
            wv_sb = [sb.tile([128, AD], DTR, name=f"wv{i}", tag="w", bufs=24) for i in range(HT)]
            for i in range(HT):
                nc.sync.dma_start(wq_sb[i][:], wq[i * 128 : (i + 1) * 128, :])
                nc.sync.dma_start(wk_sb[i][:], wk[i * 128 : (i + 1) * 128, :])
                nc.sync.dma_start(wv_sb[i][:], wv[i * 128 : (i + 1) * 128, :])
            bq_sb = sb.tile([AD, 1], DT)
            bk_sb = sb.tile([AD, 1], DT)
            bv_sb = sb.tile([AD, 1], DT)
            nc.sync.dma_start(bq_sb[:], bq[:])
            nc.sync.dma_start(bk_sb[:], bk[:])
            nc.sync.dma_start(bv_sb[:], bv[:])

            wo_sb = [sb.tile([128, HIDDEN], DTR, name=f"wo{i}", tag="wo", bufs=HT) for i in range(HT)]
            for i in range(HT):
                nc.sync.dma_start(wo_sb[i][:], wo[i * 128 : (i + 1) * 128, :])
            bo_sb = sb.tile([1, HIDDEN], DT)
            nc.sync.dma_start(bo_sb[:], bo[:])
            bo_bc = sb.tile([128, HIDDEN], DT)
            nc.gpsimd.partition_broadcast(bo_bc[:], bo_sb[:1, :])

            # persistent per-pair tensors
            qT = sb.tile([AD, N], DTR)  # rows: head0 d0..63, head1 d0..63
            kT = sb.tile([AD, N], DTR)
            att = sb.tile([AD, N], DTR)  # normalized attention output^T

            # V with appended ones column, natural [k, 65] per k-tile, per head
            v_nat = [
                [sb.tile([128, D + 1], DTR, name=f"vn{h}_{k}", tag="vnat", bufs=2 * NT) for k in range(NT)]
                for h in range(HPC)
            ]

            # ---- phase 1: qkv^T ----------------------------------------
            for c in range(HT):  # 8 n-chunks of 512
                cs = slice(c * 512, (c + 1) * 512)
                xt = [sb.tile([128, 512], DTR, name=f"xt{i}", tag="xt", bufs=12) for i in range(HT)]
                for i in range(HT):
                    nc.sync.dma_start(xt[i][:], xT[i * 128 : (i + 1) * 128, cs])

                for w_sb, b_sb, dst in ((wq_sb, bq_sb, qT), (wk_sb, bk_sb, kT)):
                    pq = ps.tile([128, 512], DT, name="pq", tag="ps_big")
                    for i in range(HT):
                        nc.tensor.matmul(
                            pq[:AD, :], (w_sb[i][:]), (xt[i][:]),
                            start=(i == 0), stop=(i == HT - 1),
                        )
                    nc.vector.tensor_scalar_add(dst[:, cs], pq[:AD, :], b_sb[:])

                pv = ps.tile([128, 512], DT, name="pv", tag="ps_big")
                for i in range(HT):
                    nc.tensor.matmul(
                        pv[:AD, :], (wv_sb[i][:]), (xt[i][:]),
                        start=(i == 0), stop=(i == HT - 1),
                    )
                v_sb = sb.tile([AD, 512], DT, name="v_sb", tag="vsb", bufs=2)
                nc.vector.tensor_scalar_add(v_sb[:], pv[:AD, :], bv_sb[:])
                # transpose v^T chunk into natural [k,64] tiles (+ ones col)
                for h in range(HPC):
                    hs = slice(h * D, (h + 1) * D)
                    for j in range(4):
                        kt_i = c * 4 + j
                        pt = ps.tile([128, D], DT, name="pt", tag="ps_big")
                        nc.tensor.matmul(
                            pt[:], v_sb[hs, j * 128 : (j + 1) * 128],
                            ident[hs, hs], is_transpose=True,
                        )
                        vn = v_nat[h][kt_i]
                        nc.vector.tensor_copy(vn[:, :D], pt[:])
                        # memset rejects float32r; 1.0f bit pattern via uint32
                        nc.vector.memset(
                            vn[:, D : D + 1].bitcast(mybir.dt.uint32), 0x3F800000
                        )

            # ---- phase 2: attention ------------------------------------
            for h in range(HPC):
                hs = slice(h * D, (h + 1) * D)
                for qc in range(NQC):
                    qs = slice(qc * QCHUNK, (qc + 1) * QCHUNK)
                    acc = ps.tile([128, QCHUNK], DT, name="acc", tag="ps_acc")
                    for kt_i in range(NT):
                        s_ps = ps.tile([128, QCHUNK], DT, name="s_ps", tag="ps_big")
                        for half in range(2):
                            qh = slice(
                                qc * QCHUNK + half * 512,
                                qc * QCHUNK + (half + 1) * 512,
                            )
                            nc.tensor.matmul(
                                s_ps[:, half * 512 : (half + 1) * 512],
                                (kT[hs, kt_i * 128 : (kt_i + 1) * 128]),
                                (qT[hs, qh]),
                                start=True, stop=True,
                            )
                        p_sb = sb.tile([128, QCHUNK], DTR, name="p_sb", tag="p_sb", bufs=2)
                        nc.scalar.activation(p_sb[:], s_ps[:], AF.Exp, scale=0.125)
                        for half in range(2):
                            hsl = slice(half * 512, (half + 1) * 512)
                            nc.tensor.matmul(
                                acc[: D + 1, hsl],
                                (v_nat[h][kt_i][:]),
                                (p_sb[:, hsl]),
                                start=(kt_i == 0), stop=(kt_i == NT - 1),
                            )
                    # normalize: att[hs, qs] = acc[:64] * (1/acc[64]) bcast
                    recip = sb.tile([1, QCHUNK], DT, name="recip", tag="recip", bufs=1)
                    nc.vector.reciprocal(recip[:], acc[D : D + 1, :])
                    bcast = sb.tile([D, QCHUNK], DT, name="bcast", tag="bcast", bufs=1)
                    nc.gpsimd.partition_broadcast(bcast[:], recip[:1, :])
                    nc.vector.tensor_mul(att[hs, qs], acc[:D, :], bcast[:])

            # ---- phase 3: all-to-all (head-shard -> seq-shard) ---------
            a2a_in = dram.tile([N_CORES, AD, NSLICE], DTR)
            a2a_out = dram.tile([N_CORES, AD, NSLICE], DTR)
            for j in range(N_CORES):
                nc.sync.dma_start(
                    a2a_in[j], att[:, j * NSLICE : (j + 1) * NSLICE]
                )
            nc.gpsimd.collective_compute(
                "AllToAll",
                mybir.AluOpType.bypass,
                replica_groups=[list(range(N_CORES))],
                ins=[a2a_in.opt()],
                outs=[a2a_out.opt()],
            )

            # ---- phase 4: output projection on my 512 rows -------------
            aT = [sb.tile([128, NSLICE], DTR, name=f"aT{i}", tag="aT", bufs=HT) for i in range(HT)]
            for i in range(HT):
                nc.sync.dma_start(aT[i][:], a2a_out[i])
            for nt in range(NSLICE // 128):
                ns = slice(nt * 128, (nt + 1) * 128)
                for cc in range(2):
                    os_ = slice(cc * 512, (cc + 1) * 512)
                    po = ps.tile([128, 512], DT, name="po", tag="ps_big")
                    for i in range(HT):
                        nc.tensor.matmul(
                            po[:], (aT[i][:, ns]), (wo_sb[i][:, os_]),
                            start=(i == 0), stop=(i == HT - 1),
                        )
                    out_sb = sb.tile([128, 512], DT, name="out_sb", tag="out_sb", bufs=2)
                    nc.vector.tensor_add(out_sb[:], po[:], bo_bc[:, os_])
                    nc.sync.dma_start(out[ns, os_], out_sb[:])

    nc.compile()
    return nc


def _get_nc(mm_mode: str):
    if mm_mode not in _CACHE:
        _CACHE[mm_mode] = _build(mm_mode)
    return _CACHE[mm_mode]


def make_in_maps(x, w_qkv, b_qkv, w_out, b_out):
    x = np.asarray(x, dtype=np.float32)
    w_qkv = np.asarray(w_qkv, dtype=np.float32)
    b_qkv = np.asarray(b_qkv, dtype=np.float32)
    w_out = np.asarray(w_out, dtype=np.float32)
    b_out = np.asarray(b_out, dtype=np.float32)

    xT = np.ascontiguousarray(x.reshape(N, HIDDEN).T)
    in_maps = []
    for c in range(N_CORES):
        cs = slice(c * AD, (c + 1) * AD)
        in_maps.append(
            {
                "xT": xT,
                "wq": np.ascontiguousarray(w_qkv[:, :HIDDEN][:, cs]),
                "wk": np.ascontiguousarray(w_qkv[:, HIDDEN : 2 * HIDDEN][:, cs]),
                "wv": np.ascontiguousarray(w_qkv[:, 2 * HIDDEN :][:, cs]),
                "bq": np.ascontiguousarray(b_qkv[:HIDDEN][cs].reshape(AD, 1)),
                "bk": np.ascontiguousarray(b_qkv[HIDDEN : 2 * HIDDEN][cs].reshape(AD, 1)),
                "bv": np.ascontiguousarray(b_qkv[2 * HIDDEN :][cs].reshape(AD, 1)),
                "wo": np.ascontiguousarray(w_out),
                "bo": np.ascontiguousarray(b_out.reshape(1, HIDDEN)),
            }
        )
    return in_maps


def kernel(x, w_qkv, b_qkv, w_out, b_out):
    from concourse.bass_utils import run_bass_kernel_spmd

    mm_mode = os.environ.get("TRN_MM_MODE", "f32r")
    nc = _get_nc(mm_mode)
    in_maps = make_in_maps(x, w_qkv, b_qkv, w_out, b_out)
    res = run_bass_kernel_spmd(nc, in_maps, list(range(N_CORES)))
    full = np.concatenate([res.results[c]["out"] for c in range(N_CORES)], axis=0)
    return full.reshape(1, N, HIDDEN).astype(np.float32)


# revision 19
# speedup vs baseline: 1.5210x; 1.5210x over previous
"""Trainium2 Bass kernel for a 16-head dense attention layer.

Problem: x[1,4096,1024] @ w_qkv[1024,3072] -> 16-head attention (N=4096,
D=64) -> @ w_out[1024,1024].

Sharding: tensor-parallel over heads across 8 NeuronCores (2 heads/core).
Each core computes q/k/v^T for its 2 heads (weights column-sliced on host),
attention with a fused, max-free softmax (scores are bounded, inputs are
xavier/randn scaled, so exp never overflows in fp32; denominator comes from
an appended ones-column in V), then an AllToAll converts the head-sharded
attention output into a sequence-sharded layout so every core applies the
full output projection to its own 512 rows. Host concatenates the 8 row
slices.

Matmuls run as float32r (fp32 data reinterpreted; 1 PE cycle/row at free
dim >= 256) by default; set TRN_MM_MODE=f32 for full-precision fp32 (4x
slower).
"""

import os
import numpy as np

N_CORES = 8
N = 4096
HIDDEN = 1024
D = 64
HPC = 2  # heads per core
AD = HPC * D  # 128 att-dim rows per core
NT = N // 128  # 32 n-tiles of 128 (k-tiles of attention)
HT = HIDDEN // 128  # 8 hidden tiles
QCHUNK = 1024
NQC = N // QCHUNK  # 4 q-chunks
NSLICE = N // N_CORES  # 512 rows of output per core

_CACHE = {}


def _build(mm_mode: str, use_collective: bool = True, att_nt: int = NT, skip_a2a: bool = False):
    import concourse.bass as bass
    import concourse.mybir as mybir
    import concourse.tile as tile
    from concourse import bacc
    from concourse.masks import make_identity

    DT = mybir.dt.float32
    DTR = {"f32r": mybir.dt.float32r, "f32": mybir.dt.float32}[mm_mode]

    AF = mybir.ActivationFunctionType

    nc = bacc.Bacc("TRN2", debug=False, num_devices=N_CORES)

    xT = nc.dram_tensor("xT", [HIDDEN, N], DTR, kind="ExternalInput").ap()
    wq = nc.dram_tensor("wq", [HIDDEN, AD], DTR, kind="ExternalInput").ap()
    wk = nc.dram_tensor("wk", [HIDDEN, AD], DTR, kind="ExternalInput").ap()
    wv = nc.dram_tensor("wv", [HIDDEN, AD], DTR, kind="ExternalInput").ap()
    bq = nc.dram_tensor("bq", [AD, 1], DT, kind="ExternalInput").ap()
    bk = nc.dram_tensor("bk", [AD, 1], DT, kind="ExternalInput").ap()
    bv = nc.dram_tensor("bv", [AD, 1], DT, kind="ExternalInput").ap()
    wo = nc.dram_tensor("wo", [HIDDEN, HIDDEN], DTR, kind="ExternalInput").ap()
    bo = nc.dram_tensor("bo", [1, HIDDEN], DT, kind="ExternalInput").ap()
    out = nc.dram_tensor("out", [NSLICE, HIDDEN], DT, kind="ExternalOutput").ap()

    with tile.TileContext(nc) as tc:
        with (
            tc.tile_pool(name="sb", bufs=1) as sb,
            tc.tile_pool(name="ps", bufs=2, space="PSUM") as ps,
            tc.tile_pool(name="dram", bufs=1, space="DRAM") as dram,
        ):
            # Global reordering: the sequence axis n is processed in
            # "stripe" order n' = (m, j, t) <-> n = 512*j + 128*m + t
            # (m: stripe 0..3, j: destination core 0..7, t: 0..127).
            # Attention is permutation-invariant in the key axis as long as
            # k and v use the same order, and the q axis just needs the
            # inverse map applied at output -- which the AllToAll block
            # routing does implicitly. Stripe m's attention output IS the
            # m-th out-row-tile of every core, so each stripe's AllToAll +
            # out-projection pipeline behind the next stripe's attention.
            ident = sb.tile([128, 128], DT)
            make_identity(nc, ident)

            # qkv weights: one DMA each, [1024, 128] folded to [128, 8*128]
            wq_sb = sb.tile([128, HT * AD], DTR)
            wk_sb = sb.tile([128, HT * AD], DTR)
            wv_sb = sb.tile([128, HT * AD], DTR)
            for w_sb, wsrc in ((wq_sb, wq), (wk_sb, wk), (wv_sb, wv)):
                nc.sync.dma_start(
                    w_sb[:].rearrange("p (a c) -> p a c", a=HT),
                    wsrc.rearrange("(a p) c -> p a c", p=128),
                )
            bq_sb = sb.tile([AD, 1], DT)
            bk_sb = sb.tile([AD, 1], DT)
            bv_sb = sb.tile([AD, 1], DT)
            nc.sync.dma_start(bq_sb[:], bq[:])
            nc.sync.dma_start(bk_sb[:], bk[:])
            nc.sync.dma_start(bv_sb[:], bv[:])

            def wslice(w_sb, i):
                return w_sb[:, i * AD : (i + 1) * AD]

            # Host pre-permutes x columns into stripe order n' = (m, j, t),
            # so streaming, qT, kTc, v_nat are all plain contiguous in n'.
            qT = sb.tile([AD, N], DTR)
            kTc = [sb.tile([AD, 512], DTR, name=f"kTc{c}", tag="kTc", bufs=HT) for c in range(HT)]
            att_m = [sb.tile([AD, QCHUNK], DTR, name=f"attm{m}", tag="attm", bufs=NQC) for m in range(NQC)]
            v_nat = [
                [sb.tile([128, D + 1], DTR, name=f"vn{h}_{k}", tag="vnat", bufs=2 * NT) for k in range(NT)]
                for h in range(HPC)
            ]
            wo_sb = [sb.tile([128, HIDDEN], DTR, name=f"wo{i}", tag="wo", bufs=HT) for i in range(HT)]
            bo_bc = sb.tile([128, HIDDEN], DT)

            a2a_in = [dram.tile([N_CORES, AD, 2, 128], DTR, name=f"a2ai{g}", tag="a2ai", bufs=2) for g in range(2)]
            a2a_out = [dram.tile([N_CORES, AD, 2, 128], DTR, name=f"a2ao{g}", tag="a2ao", bufs=2) for g in range(2)]

            # ---- emission helpers --------------------------------------
            def emit_chunk(cp):
                """phase-1 qkv^T for stream chunk cp (stripe cp//2, j-half cp%2)."""
                cs = slice(cp * 512, (cp + 1) * 512)
                xt = [sb.tile([128, 4 * 512], DTR, name=f"xt{g}", tag="xt", bufs=4) for g in range(2)]
                for g in range(2):
                    nc.sync.dma_start(
                        xt[g][:].rearrange("p (a t) -> p a t", a=4),
                        xT[g * 512 : (g + 1) * 512, cs].rearrange("(a p) t -> p a t", p=128),
                    )

                def xslice(i):
                    return xt[i // 4][:, (i % 4) * 512 : (i % 4 + 1) * 512]

                for w_sb, b_sb, dst in (
                    (wq_sb, bq_sb, qT[:, cs]),
                    (wk_sb, bk_sb, kTc[cp][:]),
                ):
                    pq = ps.tile([128, 512], DT, name="pq", tag="ps_big")
                    for i in range(HT):
                        nc.tensor.matmul(
                            pq[:AD, :], wslice(w_sb, i), xslice(i),
                            start=(i == 0), stop=(i == HT - 1),
                        )
                    nc.vector.tensor_scalar_add(dst, pq[:AD, :], b_sb[:])

                pv = ps.tile([128, 512], DT, name="pv", tag="ps_big")
                for i in range(HT):
                    nc.tensor.matmul(
                        pv[:AD, :], wslice(wv_sb, i), xslice(i),
                        start=(i == 0), stop=(i == HT - 1),
                    )
                v_sb = sb.tile([AD, 512], DT, name="v_sb", tag="vsb", bufs=2)
                nc.vector.tensor_scalar_add(v_sb[:], pv[:AD, :], bv_sb[:])
                for h in range(HPC):
                    hs = slice(h * D, (h + 1) * D)
                    for j in range(4):
                        kt_i = cp * 4 + j
                        pt = ps.tile([128, D], DT, name="pt", tag="ps_big")
                        nc.tensor.matmul(
                            pt[:], v_sb[hs, j * 128 : (j + 1) * 128],
                            ident[hs, hs], is_transpose=True,
                        )
                        vn = v_nat[h][kt_i]
                        nc.vector.tensor_copy(vn[:, :D], pt[:])
                        # memset rejects float32r; 1.0f bit pattern via uint32
                        nc.vector.memset(
                            vn[:, D : D + 1].bitcast(mybir.dt.uint32), 0x3F800000
                        )

            def emit_att_kt(m, kt_i, h, accs, sps):
                hs = slice(h * D, (h + 1) * D)
                s_ps = ps.tile([128, QCHUNK], DT, name="s_ps", tag="ps_big")
                for half in range(2):
                    nc.tensor.matmul(
                        s_ps[:, half * 512 : (half + 1) * 512],
                        kTc[kt_i // 4][hs, (kt_i % 4) * 128 : (kt_i % 4 + 1) * 128],
                        qT[hs, m * QCHUNK + half * 512 : m * QCHUNK + (half + 1) * 512],
                        start=True, stop=True,
                    )
                p_sb = sb.tile([128, QCHUNK], DTR, name="p_sb", tag="p_sb", bufs=3)
                nc.scalar.activation(p_sb[:], s_ps[:], AF.Exp, scale=0.125)
                for half in range(2):
                    hsl = slice(half * 512, (half + 1) * 512)
                    nc.tensor.matmul(
                        accs[h][: D + 1, hsl],
                        v_nat[h][kt_i][:],
                        p_sb[:, hsl],
                        start=(kt_i == 0), stop=(kt_i == NT - 1),
                    )

            def emit_finish_stripe(m, accs):
                for h in range(HPC):
                    hs = slice(h * D, (h + 1) * D)
                    recip = sb.tile([1, QCHUNK], DT, name="recip", tag="recip", bufs=2)
                    nc.vector.reciprocal(recip[:], accs[h][D : D + 1, :])
                    bcast = sb.tile([D, QCHUNK], DT, name="bcast", tag="bcast", bufs=2)
                    nc.gpsimd.partition_broadcast(bcast[:], recip[:1, :])
                    nc.vector.tensor_mul(att_m[m][hs, :], accs[h][:D, :], bcast[:])
                g, s = m // 2, m % 2
                for j in range(N_CORES):
                    nc.sync.dma_start(
                        a2a_in[g][j, :, s, :], att_m[m][:, j * 128 : (j + 1) * 128]
                    )
                if s == 1 and not skip_a2a:
                    nc.gpsimd.collective_compute(
                        "AllToAll",
                        mybir.AluOpType.bypass,
                        replica_groups=[list(range(N_CORES))],
                        ins=[a2a_in[g].opt()],
                        outs=[a2a_out[g].opt()],
                    )

            def emit_aTm_load(m):
                aTm = sb.tile([128, N_CORES * 128], DTR, name="aTm", tag="aTm", bufs=4)
                nc.sync.dma_start(
                    aTm[:].rearrange("p (a t) -> p a t", a=N_CORES),
                    a2a_out[m // 2][:, :, m % 2, :].rearrange("a p t -> p a t"),
                )
                return aTm

            def emit_outproj_cc(m, aTm, cc):
                os_ = slice(cc * 512, (cc + 1) * 512)
                po = ps.tile([128, 512], DT, name="po", tag="ps_big")
                for i in range(HT):
                    nc.tensor.matmul(
                        po[:], aTm[:, i * 128 : (i + 1) * 128], wo_sb[i][:, os_],
                        start=(i == 0), stop=(i == HT - 1),
                    )
                out_sb = sb.tile([128, 512], DT, name="out_sb", tag="out_sb", bufs=2)
                nc.vector.tensor_add(out_sb[:], po[:], bo_bc[:, os_])
                nc.sync.dma_start(out[m * 128 : (m + 1) * 128, os_], out_sb[:])

            # ---- schedule ----------------------------------------------
            def new_accs():
                return [
                    ps.tile([128, QCHUNK], DT, name=f"acc{h}", tag="ps_acc")
                    for h in range(HPC)
                ]

            # phase 1 chunks with stripe-0 attention interleaved: stripe 0's
            # q is stream chunks 0-1; its kt tiles arrive with each chunk.
            emit_chunk(0)
            emit_chunk(1)
            accs = new_accs()
            for cp in range(2, HT):
                emit_chunk(cp)
                for kt_i in range(4 * (cp - 2), 4 * (cp - 1)):
                    for h in range(HPC):
                        emit_att_kt(0, kt_i, h, accs, None)
            # out-proj weights load during attention (DMA otherwise idle)
            for i in range(HT):
                nc.sync.dma_start(wo_sb[i][:], wo[i * 128 : (i + 1) * 128, :])
            bo_sb = sb.tile([1, HIDDEN], DT)
            nc.sync.dma_start(bo_sb[:], bo[:])
            nc.gpsimd.partition_broadcast(bo_bc[:], bo_sb[:1, :])
            for kt_i in range(4 * (HT - 2), att_nt):
                for h in range(HPC):
                    emit_att_kt(0, kt_i, h, accs, None)
            emit_finish_stripe(0, accs)

            for m in range(1, NQC):
                accs = new_accs()
                prev_aTm = None
                for kt_i in range(att_nt):
                    # stripes 0/1's out-projection rides inside stripes 2/3
                    if m >= 2:
                        if kt_i == 10:
                            prev_aTm = emit_aTm_load(m - 2)
                        elif kt_i == 16:
                            emit_outproj_cc(m - 2, prev_aTm, 0)
                        elif kt_i == 24:
                            emit_outproj_cc(m - 2, prev_aTm, 1)
                    for h in range(HPC):
                        emit_att_kt(m, kt_i, h, accs, None)
                emit_finish_stripe(m, accs)
            for m in range(NQC - 2, NQC):
                aTm = emit_aTm_load(m)
                for cc in range(2):
                    emit_outproj_cc(m, aTm, cc)

    nc.compile()
    return nc


def _get_nc(mm_mode: str):
    if mm_mode not in _CACHE:
        _CACHE[mm_mode] = _build(mm_mode)
    return _CACHE[mm_mode]


def make_in_maps(x, w_qkv, b_qkv, w_out, b_out):
    x = np.asarray(x, dtype=np.float32)
    w_qkv = np.asarray(w_qkv, dtype=np.float32)
    b_qkv = np.asarray(b_qkv, dtype=np.float32)
    w_out = np.asarray(w_out, dtype=np.float32)
    b_out = np.asarray(b_out, dtype=np.float32)

    xT = x.reshape(N, HIDDEN).T  # [hidden, n]
    # permute n into stripe order n' = (m, j, t) <-> n = 512*j + 128*m + t
    xT = np.ascontiguousarray(
        xT.reshape(HIDDEN, N_CORES, NQC, 128).transpose(0, 2, 1, 3).reshape(HIDDEN, N)
    )
    in_maps = []
    for c in range(N_CORES):
        cs = slice(c * AD, (c + 1) * AD)
        in_maps.append(
            {
                "xT": xT,
                "wq": np.ascontiguousarray(w_qkv[:, :HIDDEN][:, cs]),
                "wk": np.ascontiguousarray(w_qkv[:, HIDDEN : 2 * HIDDEN][:, cs]),
                "wv": np.ascontiguousarray(w_qkv[:, 2 * HIDDEN :][:, cs]),
                "bq": np.ascontiguousarray(b_qkv[:HIDDEN][cs].reshape(AD, 1)),
                "bk": np.ascontiguousarray(b_qkv[HIDDEN : 2 * HIDDEN][cs].reshape(AD, 1)),
                "bv": np.ascontiguousarray(b_qkv[2 * HIDDEN :][cs].reshape(AD, 1)),
                "wo": np.ascontiguousarray(w_out),
                "bo": np.ascontiguousarray(b_out.reshape(1, HIDDEN)),
            }
        )
    return in_maps


def kernel(x, w_qkv, b_qkv, w_out, b_out):
    from concourse.bass_utils import run_bass_kernel_spmd

    mm_mode = os.environ.get("TRN_MM_MODE", "f32r")
    nc = _get_nc(mm_mode)
    in_maps = make_in_maps(x, w_qkv, b_qkv, w_out, b_out)
    res = run_bass_kernel_spmd(nc, in_maps, list(range(N_CORES)))
    full = np.concatenate([res.results[c]["out"] for c in range(N_CORES)], axis=0)
    return full.reshape(1, N, HIDDEN).astype(np.float32)
